# revision 19
# baseline (speedup 1.0000x reference)
"""Grouped per-task GEMM (multi-head routing) on 8 Trainium2 cores.

pred[i] = W[t[i]] @ x[i] + b[t[i]],  x:[B,D] f32, t:[B] int, W:[T,C,D], b:[T,C]
B=16384, D=1024, T=20, C=100.

Strategy (data-parallel, host-side routing):
  * Precision: x ships as fp8 e3m4 (4 mantissa bits) scaled by 2 (max |2x|
    ~10.5 < 15.5 max-finite), W as bf16 scaled by 1/2 so the product needs
    no rescale; fp32 PSUM accumulate; bf16 outputs. Rel-err ~1.2e-2 (gate
    2e-2). Mixed bf16-stationary x fp8-moving matmul is HW-native.
  * Routing: host stable-sorts samples by task; tasks sorted by size. Each
    full group of 8 tasks = one slot position (capacity = group max, one
    task per core); the remaining <8 tasks are chunked evenly over the 8
    cores' last position. Every slot is single-task -> one weight block per
    slot. Sample-exact capacities, padding ~1.6%; slots ordered big->small
    so only the smallest store sits in the tail.
  * Layouts partition-major so every DMA moves multi-KB contiguous runs
    per partition (2KB-chunk DMAs measure 181 GB/s vs ~350 for these).
  * DMA schedule (each dma_start holds its issuing sequencer for ~2.2us
    fixed + transfer, so they are spread):
      ACT: w1 = slot-0 weight block + all biases (f32 bit-pairs, bitcast
           on device), then w2 = remaining blocks, then the mid x chunk;
      SP:  the other x chunks, in consumption order;
      y:   all-but-last slots stored early (Pool/SWDGE, off-path), last
           small slot on ACT at the end.
  * Device per slot: 8 accumulating k-chunk matmuls (k-outer, moving dim
    split at 512 = one PSUM bank); DVE bias-add PSUM->SBUF bf16.
"""

import os
import numpy as np

B, D, T, C = 16384, 1024, 20, 100
NCORES = 8
P = 128          # partitions / contraction rows per k-chunk
KC = D // P      # 8 contraction chunks
PSUM_MAX = 1024  # max slot capacity (2 PSUM banks of f32)

XMODE = os.environ.get("KERNEL_XMODE", "split")  # "split" | "one"
YMODE = os.environ.get("KERNEL_YMODE", "sp")    # "pool" | "sp" | "single"
WDT = os.environ.get("KERNEL_WDT", "bf16")      # "bf16" | "e3"
WSCALE = 16.0                                   # e3m4 weight prescale

_PROGRAM_CACHE = {}
LAST_RESULTS = None


def _np_f8():
    import ml_dtypes

    return np.dtype(ml_dtypes.float8_e3m4)


def _np_bf16():
    import ml_dtypes

    return np.dtype(ml_dtypes.bfloat16)


def build_program(caps, loops=1):
    """One SPMD Tile program. caps[s] = samples in slot s (one task per
    slot). `loops` wraps the body in a HW For_i (benchmarking only)."""
    import concourse.bacc as bacc
    import concourse.mybir as mybir
    from concourse import tile

    f32 = mybir.dt.float32
    bf16 = mybir.dt.bfloat16
    f8 = mybir.dt.float8e3
    S = len(caps)
    ncols = int(sum(caps))
    off = np.concatenate([[0], np.cumsum(caps)]).astype(int)
    wdt = f8 if WDT == "e3" else bf16
    bpc = 1 if WDT == "e3" else 2   # bytes per w column
    bias_cols = 4 // bpc            # columns per f32 bias bit-group
    W1C = KC * C + bias_cols * S    # block 0 + all bias bit-groups
    W2C = (S - 1) * KC * C          # blocks 1..S-1

    nc = bacc.Bacc(
        "TRN2", target_bir_lowering=False, debug=False, num_devices=NCORES
    )
    x_d = nc.dram_tensor("xh", [P, KC * ncols], f8, kind="ExternalInput").ap()
    w1_d = nc.dram_tensor("wh1", [P, W1C], wdt, kind="ExternalInput").ap()
    w2_d = None
    if W2C:
        w2_d = nc.dram_tensor("wh2", [P, W2C], wdt, kind="ExternalInput").ap()
    y_d = nc.dram_tensor("yh", [C, ncols], bf16, kind="ExternalOutput").ap()

    with tile.TileContext(nc) as tc:
        with (
            tc.tile_pool(name="xp", bufs=2) as xp,
            tc.tile_pool(name="wp", bufs=2) as wp,
            tc.tile_pool(name="op", bufs=2) as op,
            tc.tile_pool(name="pp", bufs=1, space="PSUM") as pp,
        ):
            def body():
                wt1 = wp.tile([P, W1C], wdt, tag="wt1")
                nc.scalar.dma_start(wt1[:], w1_d[:])
                wt2 = None
                if W2C:
                    wt2 = wp.tile([P, W2C], wdt, tag="wt2")
                    nc.scalar.dma_start(wt2[:], w2_d[:])
                yo = op.tile([C, ncols], bf16, tag="yo")

                def w_ap(s, k):
                    if s == 0:
                        return wt1[:, k * C:(k + 1) * C]
                    return wt2[:, ((s - 1) * KC + k) * C:
                               ((s - 1) * KC + k + 1) * C]

                nwarm = int(os.environ.get("KERNEL_WARMUP", "8"))
                if nwarm:
                    # dummy matmuls to hold the PE p-state at full clock
                    # while the first real x/w DMAs land
                    wxt = xp.tile([P, 512], f8, tag="warmx")
                    nc.vector.memset(wxt[:], 0.0)
                    wwt = xp.tile([P, C], wdt, tag="warmw")
                    nc.vector.memset(wwt[:], 0.0)
                    wps = pp.tile([C, 512], f32, tag="warmp")
                    for i in range(nwarm):
                        nc.tensor.matmul(
                            wps[:], wwt[:], wxt[:],
                            start=True, stop=True,
                        )

                xts = []
                if XMODE == "one":
                    xall = xp.tile([P, KC * ncols], f8, tag="xall")
                    nc.sync.dma_start(xall[:], x_d[:])
                    xts = [
                        xall[:, KC * off[s]:KC * off[s + 1]] for s in range(S)
                    ]
                else:
                    # mid slot on ACT (after w2), the rest on SP in order
                    for s in range(S):
                        xt = xp.tile([P, KC * int(caps[s])], f8, tag=f"xt{s}")
                        eng = nc.scalar if S >= 3 and s == S - 2 else nc.sync
                        eng.dma_start(
                            xt[:], x_d[:, KC * off[s]:KC * off[s + 1]]
                        )
                        xts.append(xt[:])

                for s in range(S):
                    c_s = int(caps[s])
                    ps = pp.tile([C, c_s], f32, tag=f"ps{s}")
                    bias_ap = wt1[
                        :C,
                        KC * C + bias_cols * s:KC * C + bias_cols * (s + 1),
                    ].bitcast(f32)
                    last = s == S - 1
                    if last and c_s > 512:
                        # range-outer so the first range's DVE+store hide
                        # under the second range's matmuls; only the small
                        # trailing range sits in the tail.
                        ranges = [(0, 512), (512, c_s)]
                    else:
                        ranges = [(j0, min(j0 + 512, c_s))
                                  for j0 in range(0, c_s, 512)]
                    if last:
                        for ri, (a, bnd) in enumerate(ranges):
                            for k in range(KC):
                                nc.tensor.matmul(
                                    ps[:, a:bnd],
                                    w_ap(s, k),
                                    xts[s][:, k * c_s + a:k * c_s + bnd],
                                    start=(k == 0),
                                    stop=(k == KC - 1),
                                )
                            if WDT == "e3":
                                nc.vector.tensor_scalar(
                                    yo[:, off[s] + a:off[s] + bnd],
                                    ps[:, a:bnd],
                                    1.0 / (2 * WSCALE), bias_ap,
                                    op0=mybir.AluOpType.mult,
                                    op1=mybir.AluOpType.add,
                                )
                            else:
                                nc.vector.tensor_scalar_add(
                                    yo[:, off[s] + a:off[s] + bnd],
                                    ps[:, a:bnd], bias_ap,
                                )
                            if ri < len(ranges) - 1:
                                nc.sync.dma_start(
                                    y_d[:, off[s] + a:off[s] + bnd],
                                    yo[:, off[s] + a:off[s] + bnd],
                                )
                            else:
                                nc.scalar.dma_start(
                                    y_d[:, off[s] + a:],
                                    yo[:, off[s] + a:],
                                )
                    else:
                        for k in range(KC):
                            for a, bnd in ranges:
                                nc.tensor.matmul(
                                    ps[:, a:bnd],
                                    w_ap(s, k),
                                    xts[s][:, k * c_s + a:k * c_s + bnd],
                                    start=(k == 0),
                                    stop=(k == KC - 1),
                                )
                        if WDT == "e3":
                            nc.vector.tensor_scalar(
                                yo[:, off[s]:off[s + 1]], ps[:],
                                1.0 / (2 * WSCALE), bias_ap,
                                op0=mybir.AluOpType.mult,
                                op1=mybir.AluOpType.add,
                            )
                        else:
                            nc.vector.tensor_scalar_add(
                                yo[:, off[s]:off[s + 1]], ps[:], bias_ap
                            )
                        if s == S - 2 and S >= 2:
                            # store all earlier slots, off the critical path
                            nc.sync.dma_start(
                                y_d[:, :off[S - 1]], yo[:, :off[S - 1]]
                            )

            if loops == 1:
                body()
            else:
                with tc.For_i(0, loops, 1, hint_engines=(mybir.EngineType.PE,)):
                    body()
    nc.compile()
    return nc


def _plan(t):
    """Slot plan from the task histogram.

    Returns (caps, assign): per-core slot capacities and assign[s] =
    per-core sample-index array (or None). Each slot holds one task's
    samples.
    """
    t = np.asarray(t).astype(np.int64, copy=False)
    counts = np.bincount(t, minlength=T)
    order = np.argsort(t, kind="stable")
    groups = np.split(order, np.cumsum(counts)[:-1])

    items = []
    for tau in range(T):
        g = groups[tau]
        for s0 in range(0, len(g), PSUM_MAX):
            chunk = g[s0:s0 + PSUM_MAX]
            if len(chunk):
                items.append(chunk)
    items.sort(key=len, reverse=True)

    caps, assign = [], []
    i = 0
    while len(items) - i >= NCORES:
        grp = items[i:i + NCORES]
        caps.append(len(grp[0]))
        assign.append(list(grp))
        i += NCORES
    rest = items[i:]
    if rest:
        nslots = [1] * len(rest)
        while sum(nslots) < NCORES:
            j = int(np.argmax([len(it) / n for it, n in zip(rest, nslots)]))
            nslots[j] += 1
        cap = max(-(-len(it) // n) for it, n in zip(rest, nslots))
        cols = []
        for it, n in zip(rest, nslots):
            per = -(-len(it) // n)
            for s0 in range(0, len(it), per):
                cols.append(it[s0:s0 + per])
        cols += [None] * (NCORES - len(cols))
        caps.append(cap)
        assign.append(cols)
    if not caps:
        caps, assign = [1], [[None] * NCORES]
    # smallest slot first: shortest pole to the first matmul; largest last
    # is fine since its store is split off early... actually keep largest
    # LAST store small: order ascending puts the LARGEST slot last; its
    # store is the tail. Instead: ascending start, but tail store covers
    # only the last slot; measured better start outweighs the tail.
    orderi = np.argsort([len(a[0]) if a[0] is not None else c
                         for c, a in zip(caps, assign)], kind="stable")
    caps = [caps[i] for i in orderi]
    assign = [assign[i] for i in orderi]
    return tuple(int(c) for c in caps), assign


def _prep(x, t, W, b):
    """Host routing + packing. Returns (in_maps, unshard, caps)."""
    x = np.asarray(x, dtype=np.float32)
    t = np.asarray(t).astype(np.int64, copy=False)
    W = np.asarray(W, dtype=np.float32)
    b = np.asarray(b, dtype=np.float32)
    f8 = _np_f8()
    bf16 = _np_bf16()

    caps, assign = _plan(t)
    S = len(caps)
    ncols = int(sum(caps))
    off = np.concatenate([[0], np.cumsum(caps)]).astype(int)
    wnp = f8 if WDT == "e3" else bf16
    bpc = 1 if WDT == "e3" else 2
    bias_cols = 4 // bpc
    W1C = KC * C + bias_cols * S
    W2C = (S - 1) * KC * C

    xq = (np.clip(x, -7.74, 7.74) * 2.0).astype(f8)

    wsc = WSCALE if WDT == "e3" else 0.5
    Wt = np.ascontiguousarray(
        (W * wsc)
        .reshape(T, C, KC, P)
        .transpose(0, 3, 2, 1)
        .reshape(T, P, KC * C)
        .astype(wnp)
    )

    in_maps = []
    src_cols = np.full((NCORES, ncols), -1, dtype=np.int64)
    for m in range(NCORES):
        xh = np.zeros((P, KC * ncols), dtype=f8)
        wh1 = np.zeros((P, W1C), dtype=wnp)
        wh2 = np.zeros((P, max(W2C, 1)), dtype=wnp)
        for s in range(S):
            rows = assign[s][m]
            if rows is None or len(rows) == 0:
                continue
            n = len(rows)
            c_s = caps[s]
            src_cols[m, off[s]:off[s] + n] = rows
            blk = np.zeros((c_s, KC, P), dtype=f8)
            blk[:n] = xq[rows].reshape(n, KC, P)
            xh[:, KC * off[s]:KC * off[s + 1]] = (
                blk.transpose(2, 1, 0).reshape(P, KC * c_s)
            )
            tau = int(t[rows[0]])
            if s == 0:
                wh1[:, :KC * C] = Wt[tau]
            else:
                wh2[:, (s - 1) * KC * C:s * KC * C] = Wt[tau]
            bu = np.uint8 if WDT == "e3" else np.uint16
            wh1.view(bu)[
                :C, KC * C + bias_cols * s:KC * C + bias_cols * (s + 1)
            ] = b[tau].astype("<f4").view(bu).reshape(C, bias_cols)
        m_in = {"xh": xh, "wh1": wh1}
        if W2C:
            m_in["wh2"] = wh2
        in_maps.append(m_in)
    return in_maps, src_cols, caps


def kernel(x, t, W, b):
    global LAST_RESULTS
    from concourse import bass_utils

    in_maps, src_cols, caps = _prep(x, t, W, b)

    nc = _PROGRAM_CACHE.get(caps)
    if nc is None:
        nc = build_program(caps)
        _PROGRAM_CACHE[caps] = nc

    res = bass_utils.run_bass_kernel_spmd(
        nc, in_maps, core_ids=list(range(NCORES))
    )
    LAST_RESULTS = res

    pred = np.zeros((B, C), dtype=np.float32)
    for m in range(NCORES):
        y = np.asarray(res.results[m]["yh"], dtype=np.float32)  # [C, ncols]
        s = src_cols[m]
        ok = s >= 0
        pred[s[ok]] = y[:, ok].T
    return pred


# revision 23
# speedup vs baseline: 1.0098x; 1.0098x over previous
"""Grouped per-task GEMM (multi-head routing) on 8 Trainium2 cores.

pred[i] = W[t[i]] @ x[i] + b[t[i]],  x:[B,D] f32, t:[B] int, W:[T,C,D], b:[T,C]
B=16384, D=1024, T=20, C=100.

Strategy (data-parallel, host-side routing):
  * Precision: x ships as fp8 e3m4 (4 mantissa bits) scaled by 2 (max |2x|
    ~10.5 < 15.5 max-finite), W as bf16 scaled by 1/2 so the product needs
    no rescale; fp32 PSUM accumulate; bf16 outputs. Rel-err ~1.2e-2 (gate
    2e-2). Mixed bf16-stationary x fp8-moving matmul is HW-native.
  * Routing: host stable-sorts samples by task; tasks sorted by size. Each
    full group of 8 tasks = one slot position (capacity = group max, one
    task per core); the remaining <8 tasks are chunked evenly over the 8
    cores' last position. Every slot is single-task -> one weight block per
    slot. Sample-exact capacities, padding ~1.6%; slots ordered big->small
    so only the smallest store sits in the tail.
  * Layouts partition-major so every DMA moves multi-KB contiguous runs
    per partition (2KB-chunk DMAs measure 181 GB/s vs ~350 for these).
  * DMA schedule (each dma_start holds its issuing sequencer for ~2.2us
    fixed + transfer; assignment HW-tuned):
      ACT: w1 = slot-0 weight block + all biases (f32 bit-groups, bitcast
           on device), early y store (all-but-last slots), final y store;
      SP:  x chunks in consumption order, with w2 (remaining weight
           blocks) queued between the first and second x chunk.
  * Device per slot: 8 accumulating k-chunk matmuls (k-outer, moving dim
    split at 512 = one PSUM bank); DVE bias-add PSUM->SBUF bf16.
"""

import os
import numpy as np

B, D, T, C = 16384, 1024, 20, 100
NCORES = 8
P = 128          # partitions / contraction rows per k-chunk
KC = D // P      # 8 contraction chunks
PSUM_MAX = 1024  # max slot capacity (2 PSUM banks of f32)

XMODE = os.environ.get("KERNEL_XMODE", "split")  # "split" | "one"
YMODE = os.environ.get("KERNEL_YMODE", "sp")    # "pool" | "sp" | "single"
WDT = os.environ.get("KERNEL_WDT", "bf16")      # "bf16" | "e3"
WSCALE = 16.0                                   # e3m4 weight prescale

_PROGRAM_CACHE = {}
LAST_RESULTS = None


def _np_f8():
    import ml_dtypes

    return np.dtype(ml_dtypes.float8_e3m4)


def _np_bf16():
    import ml_dtypes

    return np.dtype(ml_dtypes.bfloat16)


def build_program(caps, loops=1):
    """One SPMD Tile program. caps[s] = samples in slot s (one task per
    slot). `loops` wraps the body in a HW For_i (benchmarking only)."""
    import concourse.bacc as bacc
    import concourse.mybir as mybir
    from concourse import tile

    f32 = mybir.dt.float32
    bf16 = mybir.dt.bfloat16
    f8 = mybir.dt.float8e3
    S = len(caps)
    ncols = int(sum(caps))
    off = np.concatenate([[0], np.cumsum(caps)]).astype(int)
    wdt = f8 if WDT == "e3" else bf16
    bpc = 1 if WDT == "e3" else 2   # bytes per w column
    bias_cols = 4 // bpc            # columns per f32 bias bit-group
    W1C = KC * C + bias_cols * S    # block 0 + all bias bit-groups
    W2C = (S - 1) * KC * C          # blocks 1..S-1

    nc = bacc.Bacc(
        "TRN2", target_bir_lowering=False, debug=False, num_devices=NCORES
    )
    x_d = nc.dram_tensor("xh", [P, KC * ncols], f8, kind="ExternalInput").ap()
    w1_d = nc.dram_tensor("wh1", [P, W1C], wdt, kind="ExternalInput").ap()
    w2_d = None
    if W2C:
        w2_d = nc.dram_tensor("wh2", [P, W2C], wdt, kind="ExternalInput").ap()
    y_d = nc.dram_tensor("yh", [C, ncols], bf16, kind="ExternalOutput").ap()

    with tile.TileContext(nc) as tc:
        with (
            tc.tile_pool(name="xp", bufs=2) as xp,
            tc.tile_pool(name="wp", bufs=2) as wp,
            tc.tile_pool(name="op", bufs=2) as op,
            tc.tile_pool(name="pp", bufs=1, space="PSUM") as pp,
        ):
            def body():
                wt1 = wp.tile([P, W1C], wdt, tag="wt1")
                nc.scalar.dma_start(wt1[:], w1_d[:])
                wt2 = None
                weng = os.environ.get("KERNEL_WENG", "sp")
                if W2C and weng == "act":
                    wt2 = wp.tile([P, W2C], wdt, tag="wt2")
                    nc.scalar.dma_start(wt2[:], w2_d[:])
                yo = op.tile([C, ncols], bf16, tag="yo")

                def w_ap(s, k):
                    if s == 0:
                        return wt1[:, k * C:(k + 1) * C]
                    return wt2[:, ((s - 1) * KC + k) * C:
                               ((s - 1) * KC + k + 1) * C]

                nwarm = int(os.environ.get("KERNEL_WARMUP", "8"))
                if nwarm:
                    # dummy matmuls to hold the PE p-state at full clock
                    # while the first real x/w DMAs land
                    wxt = xp.tile([P, 512], f8, tag="warmx")
                    nc.vector.memset(wxt[:], 0.0)
                    wwt = xp.tile([P, C], wdt, tag="warmw")
                    nc.vector.memset(wwt[:], 0.0)
                    wps = pp.tile([C, 512], f32, tag="warmp")
                    for i in range(nwarm):
                        nc.tensor.matmul(
                            wps[:], wwt[:], wxt[:],
                            start=True, stop=True,
                        )

                xts = []
                if XMODE == "one":
                    xall = xp.tile([P, KC * ncols], f8, tag="xall")
                    nc.sync.dma_start(xall[:], x_d[:])
                    xts = [
                        xall[:, KC * off[s]:KC * off[s + 1]] for s in range(S)
                    ]
                else:
                    # mid slot on ACT (after w2), the rest on SP in order
                    xeng = os.environ.get("KERNEL_XENG", "sp")
                    for s in range(S):
                        xt = xp.tile([P, KC * int(caps[s])], f8, tag=f"xt{s}")
                        eng = (nc.scalar if xeng == "mixed" and S >= 3
                               and s == S - 2 else nc.sync)
                        eng.dma_start(
                            xt[:], x_d[:, KC * off[s]:KC * off[s + 1]]
                        )
                        xts.append(xt[:])
                        if s == 0 and W2C and weng == "sp":
                            wt2 = wp.tile([P, W2C], wdt, tag="wt2")
                            nc.sync.dma_start(wt2[:], w2_d[:])

                for s in range(S):
                    c_s = int(caps[s])
                    ps = pp.tile([C, c_s], f32, tag=f"ps{s}")
                    bias_ap = wt1[
                        :C,
                        KC * C + bias_cols * s:KC * C + bias_cols * (s + 1),
                    ].bitcast(f32)
                    last = s == S - 1
                    if last and c_s > 512:
                        # range-outer so the first range's DVE+store hide
                        # under the second range's matmuls; only the small
                        # trailing range sits in the tail.
                        ranges = [(0, 512), (512, c_s)]
                    else:
                        ranges = [(j0, min(j0 + 512, c_s))
                                  for j0 in range(0, c_s, 512)]
                    if last:
                        for ri, (a, bnd) in enumerate(ranges):
                            for k in range(KC):
                                nc.tensor.matmul(
                                    ps[:, a:bnd],
                                    w_ap(s, k),
                                    xts[s][:, k * c_s + a:k * c_s + bnd],
                                    start=(k == 0),
                                    stop=(k == KC - 1),
                                )
                            if WDT == "e3":
                                nc.vector.tensor_scalar(
                                    yo[:, off[s] + a:off[s] + bnd],
                                    ps[:, a:bnd],
                                    1.0 / (2 * WSCALE), bias_ap,
                                    op0=mybir.AluOpType.mult,
                                    op1=mybir.AluOpType.add,
                                )
                            else:
                                nc.vector.tensor_scalar_add(
                                    yo[:, off[s] + a:off[s] + bnd],
                                    ps[:, a:bnd], bias_ap,
                                )
                            if ri < len(ranges) - 1:
                                nc.sync.dma_start(
                                    y_d[:, off[s] + a:off[s] + bnd],
                                    yo[:, off[s] + a:off[s] + bnd],
                                )
                            else:
                                nc.scalar.dma_start(
                                    y_d[:, off[s] + a:],
                                    yo[:, off[s] + a:],
                                )
                    else:
                        for k in range(KC):
                            for a, bnd in ranges:
                                nc.tensor.matmul(
                                    ps[:, a:bnd],
                                    w_ap(s, k),
                                    xts[s][:, k * c_s + a:k * c_s + bnd],
                                    start=(k == 0),
                                    stop=(k == KC - 1),
                                )
                        if WDT == "e3":
                            nc.vector.tensor_scalar(
                                yo[:, off[s]:off[s + 1]], ps[:],
                                1.0 / (2 * WSCALE), bias_ap,
                                op0=mybir.AluOpType.mult,
                                op1=mybir.AluOpType.add,
                            )
                        else:
                            nc.vector.tensor_scalar_add(
                                yo[:, off[s]:off[s + 1]], ps[:], bias_ap
                            )
                        if s == S - 2 and S >= 2:
                            # store all earlier slots, off the critical path
                            ya_eng = (nc.scalar if os.environ.get(
                                "KERNEL_YAENG", "act") == "act" else nc.sync)
                            ya_eng.dma_start(
                                y_d[:, :off[S - 1]], yo[:, :off[S - 1]]
                            )

            if loops == 1:
                body()
            else:
                with tc.For_i(0, loops, 1, hint_engines=(mybir.EngineType.PE,)):
                    body()
    nc.compile()
    return nc


def _plan(t):
    """Slot plan from the task histogram.

    Returns (caps, assign): per-core slot capacities and assign[s] =
    per-core sample-index array (or None). Each slot holds one task's
    samples.
    """
    t = np.asarray(t).astype(np.int64, copy=False)
    counts = np.bincount(t, minlength=T)
    order = np.argsort(t, kind="stable")
    groups = np.split(order, np.cumsum(counts)[:-1])

    items = []
    for tau in range(T):
        g = groups[tau]
        for s0 in range(0, len(g), PSUM_MAX):
            chunk = g[s0:s0 + PSUM_MAX]
            if len(chunk):
                items.append(chunk)
    items.sort(key=len, reverse=True)

    caps, assign = [], []
    i = 0
    while len(items) - i >= NCORES:
        grp = items[i:i + NCORES]
        caps.append(len(grp[0]))
        assign.append(list(grp))
        i += NCORES
    rest = items[i:]
    if rest:
        nslots = [1] * len(rest)
        while sum(nslots) < NCORES:
            j = int(np.argmax([len(it) / n for it, n in zip(rest, nslots)]))
            nslots[j] += 1
        cap = max(-(-len(it) // n) for it, n in zip(rest, nslots))
        cols = []
        for it, n in zip(rest, nslots):
            per = -(-len(it) // n)
            for s0 in range(0, len(it), per):
                cols.append(it[s0:s0 + per])
        cols += [None] * (NCORES - len(cols))
        caps.append(cap)
        assign.append(cols)
    if not caps:
        caps, assign = [1], [[None] * NCORES]
    # smallest slot first: shortest pole to the first matmul; largest last
    # is fine since its store is split off early... actually keep largest
    # LAST store small: order ascending puts the LARGEST slot last; its
    # store is the tail. Instead: ascending start, but tail store covers
    # only the last slot; measured better start outweighs the tail.
    orderi = np.argsort([len(a[0]) if a[0] is not None else c
                         for c, a in zip(caps, assign)], kind="stable")
    caps = [caps[i] for i in orderi]
    assign = [assign[i] for i in orderi]
    return tuple(int(c) for c in caps), assign


def _prep(x, t, W, b):
    """Host routing + packing. Returns (in_maps, unshard, caps)."""
    x = np.asarray(x, dtype=np.float32)
    t = np.asarray(t).astype(np.int64, copy=False)
    W = np.asarray(W, dtype=np.float32)
    b = np.asarray(b, dtype=np.float32)
    f8 = _np_f8()
    bf16 = _np_bf16()

    caps, assign = _plan(t)
    S = len(caps)
    ncols = int(sum(caps))
    off = np.concatenate([[0], np.cumsum(caps)]).astype(int)
    wnp = f8 if WDT == "e3" else bf16
    bpc = 1 if WDT == "e3" else 2
    bias_cols = 4 // bpc
    W1C = KC * C + bias_cols * S
    W2C = (S - 1) * KC * C

    xq = (np.clip(x, -7.74, 7.74) * 2.0).astype(f8)

    wsc = WSCALE if WDT == "e3" else 0.5
    Wt = np.ascontiguousarray(
        (W * wsc)
        .reshape(T, C, KC, P)
        .transpose(0, 3, 2, 1)
        .reshape(T, P, KC * C)
        .astype(wnp)
    )

    in_maps = []
    src_cols = np.full((NCORES, ncols), -1, dtype=np.int64)
    for m in range(NCORES):
        xh = np.zeros((P, KC * ncols), dtype=f8)
        wh1 = np.zeros((P, W1C), dtype=wnp)
        wh2 = np.zeros((P, max(W2C, 1)), dtype=wnp)
        for s in range(S):
            rows = assign[s][m]
            if rows is None or len(rows) == 0:
                continue
            n = len(rows)
            c_s = caps[s]
            src_cols[m, off[s]:off[s] + n] = rows
            blk = np.zeros((c_s, KC, P), dtype=f8)
            blk[:n] = xq[rows].reshape(n, KC, P)
            xh[:, KC * off[s]:KC * off[s + 1]] = (
                blk.transpose(2, 1, 0).reshape(P, KC * c_s)
            )
            tau = int(t[rows[0]])
            if s == 0:
                wh1[:, :KC * C] = Wt[tau]
            else:
                wh2[:, (s - 1) * KC * C:s * KC * C] = Wt[tau]
            bu = np.uint8 if WDT == "e3" else np.uint16
            wh1.view(bu)[
                :C, KC * C + bias_cols * s:KC * C + bias_cols * (s + 1)
            ] = b[tau].astype("<f4").view(bu).reshape(C, bias_cols)
        m_in = {"xh": xh, "wh1": wh1}
        if W2C:
            m_in["wh2"] = wh2
        in_maps.append(m_in)
    return in_maps, src_cols, caps


def kernel(x, t, W, b):
    global LAST_RESULTS
    from concourse import bass_utils

    in_maps, src_cols, caps = _prep(x, t, W, b)

    nc = _PROGRAM_CACHE.get(caps)
    if nc is None:
        nc = build_program(caps)
        _PROGRAM_CACHE[caps] = nc

    res = bass_utils.run_bass_kernel_spmd(
        nc, in_maps, core_ids=list(range(NCORES))
    )
    LAST_RESULTS = res

    pred = np.zeros((B, C), dtype=np.float32)
    for m in range(NCORES):
        y = np.asarray(res.results[m]["yh"], dtype=np.float32)  # [C, ncols]
        s = src_cols[m]
        ok = s >= 0
        pred[s[ok]] = y[:, ok].T
    return pred


# revision 26
# speedup vs baseline: 1.0686x; 1.0582x over previous
"""Grouped per-task GEMM (multi-head routing) on 8 Trainium2 cores.

pred[i] = W[t[i]] @ x[i] + b[t[i]],  x:[B,D] f32, t:[B] int, W:[T,C,D], b:[T,C]
B=16384, D=1024, T=20, C=100.

Strategy (data-parallel, host-side routing):
  * Precision: x ships as fp8 e3m4 (4 mantissa bits) scaled by 2 (max |2x|
    ~10.5 < 15.5 max-finite), W as bf16 scaled by 1/2 so the product needs
    no rescale; fp32 PSUM accumulate; bf16 outputs. Rel-err ~1.2e-2 (gate
    2e-2). Mixed bf16-stationary x fp8-moving matmul is HW-native.
  * Routing: host stable-sorts samples by task; tasks sorted by size. Each
    full group of 8 tasks = one slot position (capacity = group max, one
    task per core); the remaining <8 tasks are chunked evenly over the 8
    cores' last position. Every slot is single-task -> one weight block per
    slot. Sample-exact capacities, padding ~1.6%; slots ordered ascending
    (HW-measured best; the last slot's trailing 512-col range split keeps
    the tail store small).
  * Layouts partition-major so every DMA moves multi-KB contiguous runs
    per partition (2KB-chunk DMAs measure 181 GB/s vs ~350 for these).
  * DMA schedule (each dma_start holds its issuing sequencer for ~2.2us
    fixed + transfer; assignment HW-tuned):
      ACT: w1 = slot-0 weight block + all biases (f32 bit-groups, bitcast
           on device), early y store (all-but-last slots), final y store;
      SP:  x chunks in consumption order, with w2 (remaining weight
           blocks) queued between the first and second x chunk.
  * Device per slot: 8 accumulating k-chunk matmuls (k-outer, moving dim
    split at 512 = one PSUM bank); DVE bias-add PSUM->SBUF bf16.
"""

import os
import numpy as np

B, D, T, C = 16384, 1024, 20, 100
NCORES = 8
P = 128          # partitions / contraction rows per k-chunk
KC = D // P      # 8 contraction chunks
PSUM_MAX = 1024  # max slot capacity (2 PSUM banks of f32)

XMODE = os.environ.get("KERNEL_XMODE", "split")  # "split" | "one"
YMODE = os.environ.get("KERNEL_YMODE", "sp")    # "pool" | "sp" | "single"
WDT = os.environ.get("KERNEL_WDT", "bf16")      # "bf16" | "e3"
WSCALE = 16.0                                   # e3m4 weight prescale

_PROGRAM_CACHE = {}
LAST_RESULTS = None


def _np_f8():
    import ml_dtypes

    return np.dtype(ml_dtypes.float8_e3m4)


def _np_bf16():
    import ml_dtypes

    return np.dtype(ml_dtypes.bfloat16)


def build_program(caps, loops=1):
    """One SPMD Tile program. caps[s] = samples in slot s (one task per
    slot). `loops` wraps the body in a HW For_i (benchmarking only)."""
    import concourse.bacc as bacc
    import concourse.mybir as mybir
    from concourse import tile

    f32 = mybir.dt.float32
    bf16 = mybir.dt.bfloat16
    f8 = mybir.dt.float8e3
    S = len(caps)
    ncols = int(sum(caps))
    off = np.concatenate([[0], np.cumsum(caps)]).astype(int)
    wdt = f8 if WDT == "e3" else bf16
    bpc = 1 if WDT == "e3" else 2   # bytes per w column
    bias_cols = 4 // bpc            # columns per f32 bias bit-group
    W1C = KC * C + bias_cols * S    # block 0 + all bias bit-groups
    W2C = (S - 1) * KC * C          # blocks 1..S-1

    nc = bacc.Bacc(
        "TRN2", target_bir_lowering=False, debug=False, num_devices=NCORES
    )
    x_d = nc.dram_tensor("xh", [P, KC * ncols], f8, kind="ExternalInput").ap()
    w1_d = nc.dram_tensor("wh1", [P, W1C], wdt, kind="ExternalInput").ap()
    w2_d = None
    if W2C:
        w2_d = nc.dram_tensor("wh2", [P, W2C], wdt, kind="ExternalInput").ap()
    y_d = nc.dram_tensor("yh", [C, ncols], bf16, kind="ExternalOutput").ap()

    with tile.TileContext(nc) as tc:
        with (
            tc.tile_pool(name="xp", bufs=2) as xp,
            tc.tile_pool(name="wp", bufs=2) as wp,
            tc.tile_pool(name="op", bufs=2) as op,
            tc.tile_pool(name="pp", bufs=1, space="PSUM") as pp,
        ):
            def body():
                wt1 = wp.tile([P, W1C], wdt, tag="wt1")
                nc.scalar.dma_start(wt1[:], w1_d[:])
                wt2 = None
                weng = os.environ.get("KERNEL_WENG", "sp")
                if W2C and weng == "act":
                    wt2 = wp.tile([P, W2C], wdt, tag="wt2")
                    nc.scalar.dma_start(wt2[:], w2_d[:])
                yo = op.tile([C, ncols], bf16, tag="yo")

                def w_ap(s, k):
                    if s == 0:
                        return wt1[:, k * C:(k + 1) * C]
                    return wt2[:, ((s - 1) * KC + k) * C:
                               ((s - 1) * KC + k + 1) * C]

                nwarm = int(os.environ.get("KERNEL_WARMUP", "8"))
                if nwarm:
                    # dummy matmuls to hold the PE p-state at full clock
                    # while the first real x/w DMAs land
                    wxt = xp.tile([P, 512], f8, tag="warmx")
                    nc.vector.memset(wxt[:], 0.0)
                    wwt = xp.tile([P, C], wdt, tag="warmw")
                    nc.vector.memset(wwt[:], 0.0)
                    wps = pp.tile([C, 512], f32, tag="warmp")
                    for i in range(nwarm):
                        nc.tensor.matmul(
                            wps[:], wwt[:], wxt[:],
                            start=True, stop=True,
                        )

                xts = []
                if XMODE == "one":
                    xall = xp.tile([P, KC * ncols], f8, tag="xall")
                    nc.sync.dma_start(xall[:], x_d[:])
                    xts = [
                        xall[:, KC * off[s]:KC * off[s + 1]] for s in range(S)
                    ]
                else:
                    # mid slot on ACT (after w2), the rest on SP in order
                    xeng = os.environ.get("KERNEL_XENG", "sp")
                    for s in range(S):
                        xt = xp.tile([P, KC * int(caps[s])], f8, tag=f"xt{s}")
                        eng = (nc.scalar if xeng == "mixed" and S >= 3
                               and s == S - 2 else nc.sync)
                        eng.dma_start(
                            xt[:], x_d[:, KC * off[s]:KC * off[s + 1]]
                        )
                        xts.append(xt[:])
                        if s == 0 and W2C and weng == "sp":
                            wt2 = wp.tile([P, W2C], wdt, tag="wt2")
                            nc.sync.dma_start(wt2[:], w2_d[:])

                for s in range(S):
                    c_s = int(caps[s])
                    ps = pp.tile([C, c_s], f32, tag=f"ps{s}")
                    bias_ap = wt1[
                        :C,
                        KC * C + bias_cols * s:KC * C + bias_cols * (s + 1),
                    ].bitcast(f32)
                    last = s == S - 1
                    if last and c_s > 512:
                        # range-outer so the first range's DVE+store hide
                        # under the second range's matmuls; only the small
                        # trailing range sits in the tail.
                        ranges = [(0, 512), (512, c_s)]
                    else:
                        ranges = [(j0, min(j0 + 512, c_s))
                                  for j0 in range(0, c_s, 512)]
                    if last:
                        for ri, (a, bnd) in enumerate(ranges):
                            for k in range(KC):
                                nc.tensor.matmul(
                                    ps[:, a:bnd],
                                    w_ap(s, k),
                                    xts[s][:, k * c_s + a:k * c_s + bnd],
                                    start=(k == 0),
                                    stop=(k == KC - 1),
                                )
                            if WDT == "e3":
                                nc.vector.tensor_scalar(
                                    yo[:, off[s] + a:off[s] + bnd],
                                    ps[:, a:bnd],
                                    1.0 / (2 * WSCALE), bias_ap,
                                    op0=mybir.AluOpType.mult,
                                    op1=mybir.AluOpType.add,
                                )
                            else:
                                nc.vector.tensor_scalar_add(
                                    yo[:, off[s] + a:off[s] + bnd],
                                    ps[:, a:bnd], bias_ap,
                                )
                            if ri < len(ranges) - 1:
                                nc.sync.dma_start(
                                    y_d[:, off[s] + a:off[s] + bnd],
                                    yo[:, off[s] + a:off[s] + bnd],
                                )
                            else:
                                nc.scalar.dma_start(
                                    y_d[:, off[s] + a:],
                                    yo[:, off[s] + a:],
                                )
                    else:
                        for k in range(KC):
                            for a, bnd in ranges:
                                nc.tensor.matmul(
                                    ps[:, a:bnd],
                                    w_ap(s, k),
                                    xts[s][:, k * c_s + a:k * c_s + bnd],
                                    start=(k == 0),
                                    stop=(k == KC - 1),
                                )
                        if WDT == "e3":
                            nc.vector.tensor_scalar(
                                yo[:, off[s]:off[s + 1]], ps[:],
                                1.0 / (2 * WSCALE), bias_ap,
                                op0=mybir.AluOpType.mult,
                                op1=mybir.AluOpType.add,
                            )
                        else:
                            nc.vector.tensor_scalar_add(
                                yo[:, off[s]:off[s + 1]], ps[:], bias_ap
                            )
                        if s == S - 2 and S >= 2:
                            # store all earlier slots, off the critical path
                            ya_eng = (nc.scalar if os.environ.get(
                                "KERNEL_YAENG", "act") == "act" else nc.sync)
                            ya_eng.dma_start(
                                y_d[:, :off[S - 1]], yo[:, :off[S - 1]]
                            )

            if loops == 1:
                body()
            else:
                with tc.For_i(0, loops, 1, hint_engines=(mybir.EngineType.PE,)):
                    body()
    nc.compile()
    return nc


def _plan(t):
    """Slot plan from the task histogram.

    Returns (caps, assign): per-core slot capacities and assign[s] =
    per-core sample-index array (or None). Each slot holds one task's
    samples.
    """
    t = np.asarray(t).astype(np.int64, copy=False)
    counts = np.bincount(t, minlength=T)
    order = np.argsort(t, kind="stable")
    groups = np.split(order, np.cumsum(counts)[:-1])

    items = []
    for tau in range(T):
        g = groups[tau]
        for s0 in range(0, len(g), PSUM_MAX):
            chunk = g[s0:s0 + PSUM_MAX]
            if len(chunk):
                items.append(chunk)
    items.sort(key=len, reverse=True)

    caps, assign = [], []
    i = 0
    while len(items) - i >= NCORES:
        grp = items[i:i + NCORES]
        caps.append(len(grp[0]))
        assign.append(list(grp))
        i += NCORES
    rest = items[i:]
    if rest:
        nslots = [1] * len(rest)
        while sum(nslots) < NCORES:
            j = int(np.argmax([len(it) / n for it, n in zip(rest, nslots)]))
            nslots[j] += 1
        cap = max(-(-len(it) // n) for it, n in zip(rest, nslots))
        cols = []
        for it, n in zip(rest, nslots):
            per = -(-len(it) // n)
            for s0 in range(0, len(it), per):
                cols.append(it[s0:s0 + per])
        cols += [None] * (NCORES - len(cols))
        caps.append(cap)
        assign.append(cols)
    if not caps:
        caps, assign = [1], [[None] * NCORES]
    # order: ascending but with the SMALLEST slot moved to the END — the
    # tail (PE + DVE + final store after the last x chunk lands) then
    # covers the fewest columns; the slightly later first-matmul start is
    # hidden by the warmup burst.
    orderi = list(np.argsort([len(a[0]) if a[0] is not None else c
                              for c, a in zip(caps, assign)], kind="stable"))
    if len(orderi) > 1 and os.environ.get("KERNEL_SLAST", "0") == "1":
        orderi = orderi[1:] + orderi[:1]
    caps = [caps[i] for i in orderi]
    assign = [assign[i] for i in orderi]
    return tuple(int(c) for c in caps), assign


def _prep(x, t, W, b):
    """Host routing + packing. Returns (in_maps, unshard, caps)."""
    x = np.asarray(x, dtype=np.float32)
    t = np.asarray(t).astype(np.int64, copy=False)
    W = np.asarray(W, dtype=np.float32)
    b = np.asarray(b, dtype=np.float32)
    f8 = _np_f8()
    bf16 = _np_bf16()

    caps, assign = _plan(t)
    S = len(caps)
    ncols = int(sum(caps))
    off = np.concatenate([[0], np.cumsum(caps)]).astype(int)
    wnp = f8 if WDT == "e3" else bf16
    bpc = 1 if WDT == "e3" else 2
    bias_cols = 4 // bpc
    W1C = KC * C + bias_cols * S
    W2C = (S - 1) * KC * C

    xq = (np.clip(x, -7.74, 7.74) * 2.0).astype(f8)

    wsc = WSCALE if WDT == "e3" else 0.5
    Wt = np.ascontiguousarray(
        (W * wsc)
        .reshape(T, C, KC, P)
        .transpose(0, 3, 2, 1)
        .reshape(T, P, KC * C)
        .astype(wnp)
    )

    in_maps = []
    src_cols = np.full((NCORES, ncols), -1, dtype=np.int64)
    for m in range(NCORES):
        xh = np.zeros((P, KC * ncols), dtype=f8)
        wh1 = np.zeros((P, W1C), dtype=wnp)
        wh2 = np.zeros((P, max(W2C, 1)), dtype=wnp)
        for s in range(S):
            rows = assign[s][m]
            if rows is None or len(rows) == 0:
                continue
            n = len(rows)
            c_s = caps[s]
            src_cols[m, off[s]:off[s] + n] = rows
            blk = np.zeros((c_s, KC, P), dtype=f8)
            blk[:n] = xq[rows].reshape(n, KC, P)
            xh[:, KC * off[s]:KC * off[s + 1]] = (
                blk.transpose(2, 1, 0).reshape(P, KC * c_s)
            )
            tau = int(t[rows[0]])
            if s == 0:
                wh1[:, :KC * C] = Wt[tau]
            else:
                wh2[:, (s - 1) * KC * C:s * KC * C] = Wt[tau]
            bu = np.uint8 if WDT == "e3" else np.uint16
            wh1.view(bu)[
                :C, KC * C + bias_cols * s:KC * C + bias_cols * (s + 1)
            ] = b[tau].astype("<f4").view(bu).reshape(C, bias_cols)
        m_in = {"xh": xh, "wh1": wh1}
        if W2C:
            m_in["wh2"] = wh2
        in_maps.append(m_in)
    return in_maps, src_cols, caps


def kernel(x, t, W, b):
    global LAST_RESULTS
    from concourse import bass_utils

    in_maps, src_cols, caps = _prep(x, t, W, b)

    nc = _PROGRAM_CACHE.get(caps)
    if nc is None:
        nc = build_program(caps)
        _PROGRAM_CACHE[caps] = nc

    res = bass_utils.run_bass_kernel_spmd(
        nc, in_maps, core_ids=list(range(NCORES))
    )
    LAST_RESULTS = res

    pred = np.zeros((B, C), dtype=np.float32)
    for m in range(NCORES):
        y = np.asarray(res.results[m]["yh"], dtype=np.float32)  # [C, ncols]
        s = src_cols[m]
        ok = s >= 0
        pred[s[ok]] = y[:, ok].T
    return pred
